# revision 64
# baseline (speedup 1.0000x reference)
"""Trainium2 Bass kernel for nn_Attention_10282151707309.

Reference computation:
  - channel LayerNorm over C=128 (biased var, eps=1e-5, affine g/b)
  - qkv = w_qkv @ xn (1x1 conv), 4 heads x 32 dims, q scaled by 1/sqrt(32)
  - full softmax attention over HW=4096 positions per (batch, head)
  - out = w_out @ attn_out + b_out

Sharding: 8 cores = (batch b in 0..3) x (spatial half in 0..1); each core
runs an identical program on its batch slice (spatially rolled so its own
2048 query columns are program-columns 0:2048 -- softmax is permutation-
equivariant over keys). No collectives; disjoint output slices.

Performance model (from perfetto traces of earlier versions):
  - The ACT engine is the wall: exp of 33.5M sim values at 128 lanes
    @1.2GHz, ~(N+390)/1.2 ns per N-element ACTIVATE. Steady state
    achieves one [128,1024] exp per ~1.2us; everything else must stay
    off ACT and off the exp stream's critical path.
  - PE streams full-mode (128,128)-tile bf16 matmuls at ~215-258ns per
    512 columns with LDWEIGHTS hidden; partial-tile matmuls are ~1.6x
    slower, so sim/av use only full [128,x] operands.
  - Engine queues are strictly in-order: any op whose producers are not
    long-finished head-of-line blocks its whole engine. All cross-engine
    chains (LN, tails) are therefore emission-scheduled several jc
    iterations after their producers.
Structure:
  - One activation-table preload (natural_log_exp_and_others) so Ln/Exp
    never swap tables (the original baseline lost 52us to 41 loads).
  - LN + projections are emission-interleaved under i-tile 0's attention
    in 5 slots per s-tile (xb/xsq -> stats -> rstd -> bc/xn -> proj),
    each ~2 jc after its producers. s-tiles 0-1 partially in prologue.
  - sim full-mode without a padded k: lhsT is the whole k_sb chunk (all
    4 heads' rows), rhs a per-head ZERO-PADDED q copy (zeros kill the
    other heads' k rows exactly). q pads and the zero-padded av lhsT
    tiles (vaug) are zero-filled by DMA from a host zeros tensor --
    no big on-chip memsets (gpsimd per-op overhead is ~1.2-2.2us).
  - k-bias dropped entirely (softmax shift-invariance, exact); v-bias
    folded into the output bias; g/b folded into the qkv weights.
  - i-tile tails are taken OFF the critical path: pairs are spilled
    PSUM->SBUF (2 DVE copies) at it end, which immediately frees the
    pair banks for the next i-tile; the normalize/project tail runs
    against the SBUF copy, sprinkled into the next i-tile's jc loop.
PSUM (8 banks): lnps [128,1024] (2; LN generations + tail bc|y)
  + duo 2x[128,1024] (4) + pairs 2x[128,512] (2).
"""

import numpy as np

HEADS = 4
DIM_HEAD = 32
B, C, H, W = 4, 128, 64, 64
S = H * W              # 4096 spatial positions
HALF = S // 2          # 2048 own query columns per core
TI = 512               # i-tile (query) size
NIT = HALF // TI       # 4 i-tiles
JCHUNK = 128           # j-chunk (key) size
NJC = S // JCHUNK      # 32 j-chunks
EPS = 1e-5
N_CORES = 8

_PROGRAM = None


def _build_program():
    """Build the (SPMD-identical) Bass program once per process."""
    import concourse.bass as bass  # noqa: F401
    import concourse.mybir as mybir
    import concourse.tile as tile
    from concourse import bacc
    from concourse.bass import ts

    dt = mybir.dt.float32
    dtr = mybir.dt.float32r
    dtb = mybir.dt.bfloat16
    F = mybir.ActivationFunctionType
    Op = mybir.AluOpType

    nc = bacc.Bacc(
        "TRN2",
        target_bir_lowering=False,
        debug=False,
        num_devices=N_CORES,
    )

    x_d = nc.dram_tensor("x", [C, S], dt, kind="ExternalInput").ap()
    wq_d = nc.dram_tensor("wq_t", [C, 128], dtb, kind="ExternalInput").ap()
    wk_d = nc.dram_tensor("wk_t", [C, 128], dtb, kind="ExternalInput").ap()
    wv_d = nc.dram_tensor("wv_t", [C, 128], dtb, kind="ExternalInput").ap()
    woa_d = nc.dram_tensor("wo_a", [97, 128], dt, kind="ExternalInput").ap()
    wob_d = nc.dram_tensor("wo_b", [97, 128], dt, kind="ExternalInput").ap()
    bq_d = nc.dram_tensor("bias_q", [128, 1], dt, kind="ExternalInput").ap()
    bo_d = nc.dram_tensor("bias_o", [128, 1], dt, kind="ExternalInput").ap()
    z_d = nc.dram_tensor("zeros", [128, 4096], dtb, kind="ExternalInput").ap()
    y_d = nc.dram_tensor("y", [C, HALF], dt, kind="ExternalOutput").ap()

    with tile.TileContext(nc) as tc:
        from contextlib import ExitStack

        with ExitStack() as ctx:
            const_pool = ctx.enter_context(tc.tile_pool(name="const", bufs=1))
            big_pool = ctx.enter_context(tc.tile_pool(name="big", bufs=1))

            # One table set (natural_log_exp_and_others, id 6) serves every
            # activation in this kernel (Exp, Ln); preload it once.
            nc.scalar.add_instruction(
                mybir.InstLoadActFuncSet(
                    name="act_preload", act_func_set_id=6, ins=[], outs=[]
                )
            )

            wq = const_pool.tile([C, 128], dtb, tag="wq")
            wk = const_pool.tile([C, 128], dtb, tag="wk")
            wv = const_pool.tile([C, 128], dtb, tag="wv")
            woa = const_pool.tile([97, 128], dt, tag="woa")
            wob = const_pool.tile([97, 128], dt, tag="wob")
            bq = const_pool.tile([128, 1], dt, tag="bq")
            bo = const_pool.tile([128, 1], dt, tag="bo")
            ones1 = const_pool.tile([1, 128], dtb, tag="ones1")
            onesC = const_pool.tile([128, 1], dtb, tag="onesC")
            # bc lhsT: row 0 -> out rows 0:33 (even-head reciprocal), row 32
            # -> out rows 64:97 (odd-head). Engine AP partition bases must be
            # 32-aligned, so the two reciprocal rows sit at partitions 0/32.
            ones2 = const_pool.tile([33, 97], dt, tag="ones2")
            epsc = const_pool.tile([1, 1], dt, tag="epsc")

            x_sb = big_pool.tile([C, S], dt, tag="x")
            xn = big_pool.tile([C, S], dtb, tag="xn")
            k_sb = big_pool.tile([128, S], dtb, tag="k")
            # vaug: per j-chunk four [128, 128] full-mode av lhsT tiles in
            # order [h0, h2, h1, h3]; h0/h1 carry (v^T | ones) at cols 0-32,
            # h2/h3 at cols 64-96, everything else zero.
            vaug = big_pool.tile([128, NJC * 512], dtb, tag="vaug")
            q_pad = [
                big_pool.tile(
                    [128, HALF], dtb, tag=f"qpad{h}", name=f"qpad{h}"
                )
                for h in range(HEADS)
            ]
            catA = big_pool.tile([128, TI], dt, tag="catA")
            catB = big_pool.tile([128, TI], dt, tag="catB")
            rec = big_pool.tile([64, 1024], dt, tag="rec")
            # biased q staging: one [128,512] PSUM-read op per q-tile here;
            # the four per-head pad writes become cheap bf16 SBUF copies
            # emitted in uncongested zones.
            q_stage = big_pool.tile([128, HALF], dtb, tag="qstage")

            # input DMAs; zero fills come from the host zeros tensor so no
            # engine spends time on them.
            # Each dma_start trigger costs ~0.6-0.7us on the serial Sync
            # queue, so the issue ORDER is the prologue critical path:
            # x tiles for the early LN chain first, zeros last.
            nc.sync.dma_start(x_sb[:, 0:512], x_d[:, 0:512])
            nc.sync.dma_start(wq[:], wq_d[:])
            nc.sync.dma_start(wk[:], wk_d[:])
            nc.sync.dma_start(x_sb[:, 512:1024], x_d[:, 512:1024])
            nc.sync.dma_start(wv[:], wv_d[:])
            nc.sync.dma_start(bq[:], bq_d[:])
            nc.sync.dma_start(x_sb[:, 1024:1536], x_d[:, 1024:1536])
            nc.sync.dma_start(woa[:], woa_d[:])
            nc.sync.dma_start(wob[:], wob_d[:])
            nc.sync.dma_start(bo[:], bo_d[:])
            nc.sync.dma_start(x_sb[:, 1536:2048], x_d[:, 1536:2048])
            nc.sync.dma_start(vaug[:, 0:4096], z_d[:, 0:4096])
            for h in range(HEADS):
                nc.sync.dma_start(q_pad[h][:], z_d[:, 0:HALF])
            for t in range(4, 8):
                nc.sync.dma_start(x_sb[:, ts(t, 512)], x_d[:, ts(t, 512)])
            for i in range(1, 4):
                nc.sync.dma_start(
                    vaug[:, 4096 * i : 4096 * (i + 1)], z_d[:, 0:4096]
                )

            nc.vector.memset(ones1[:], 1.0)
            nc.vector.memset(onesC[:], 1.0 / C)
            nc.vector.memset(ones2[:], 0.0)
            nc.vector.memset(ones2[0:1, 0:33], 1.0)
            nc.vector.memset(ones2[32:33, 64:97], 1.0)
            nc.vector.memset(rec[:], 0.0)
            nc.vector.memset(epsc[:], EPS)
            # cat rows 33:64 are read by the K=97 y matmul (against zero
            # rows of wo) -- zero once so they are never NaN. Row 32 is
            # rewritten by every tail (32-aligned memset base).
            nc.vector.memset(catA[32:64, :], 0.0)
            nc.vector.memset(catB[32:64, :], 0.0)

            def vaug_ones(i):
                """Softmax-denominator ones columns of vaug chunk i (8 jc):
                col 32 of the h0/h1 tiles, col 96 of h2/h3. Emitted per
                chunk so each waits only its own zero-DMA."""
                blk = vaug[:, 4096 * i : 4096 * (i + 1)].rearrange(
                    "p (c g e) -> p c g e", g=2, e=256
                )
                nc.vector.memset(blk[:, :, :, 32:33], 1.0)
                nc.vector.memset(blk[:, :, :, 224:225], 1.0)

            pair_pool = ctx.enter_context(
                tc.tile_pool(name="pair_ps", bufs=2, space="PSUM")
            )
            pools = {}
            expo_pool = ctx.enter_context(tc.tile_pool(name="expo", bufs=8))
            spill_pool = ctx.enter_context(tc.tile_pool(name="spill", bufs=2))
            ysb_pool = ctx.enter_context(tc.tile_pool(name="ysb", bufs=2))
            sm_pool = ctx.enter_context(tc.tile_pool(name="lnsm", bufs=2))
            gx_pool = ctx.enter_context(tc.tile_pool(name="lngx", bufs=3))

            # ---------------- LayerNorm + projections ----------------
            # Five emission slots per s-tile; in i-tile 0 they sit ~2 jc
            # after their producers so no engine head-of-line blocks.
            ln_state = {}

            def ln_A(t):
                sl = ts(t, 512)
                xb = gx_pool.tile([128, 512], dtb, tag="xb")
                xsq = gx_pool.tile([128, 512], dtb, tag="xsq")
                nc.vector.tensor_copy(xb[:], x_sb[:, sl])
                nc.vector.tensor_tensor(xsq[:], xb[:], xb[:], Op.mult)
                ln_state[t] = (xb, xsq)

            def ln_B(t):
                xb, xsq = ln_state[t]
                # sum at partition 0, sumsq at partition 32 of ONE 1-bank
                # generation (M=1 matmuls at col tile positions 0 / 32):
                # [128,512]x2buf ring halves the serialization depth of the
                # old [128,1024]x1buf ring.
                g = pools["lnps"].tile([128, 512], dt, tag="ln")
                nc.tensor.matmul(g[0:1, :], onesC[:, 0:1], xb[:])
                nc.tensor.matmul(g[32:33, :], onesC[:, 0:1], xsq[:])
                ln_state[t] = g

            def ln_C(t):
                g = ln_state[t]
                mcp = sm_pool.tile([1, 512], dt, tag="mcp")
                msq = sm_pool.tile([1, 512], dt, tag="msq")
                var = sm_pool.tile([1, 512], dt, tag="var")
                lnv = sm_pool.tile([1, 512], dt, tag="lnv")
                ru = sm_pool.tile([1, 1024], dt, tag="ru")
                nc.vector.tensor_copy(mcp[:], g[0:1, :])
                nc.vector.tensor_tensor(msq[:], mcp[:], mcp[:], Op.mult)
                nc.vector.scalar_tensor_tensor(
                    var[:], g[32:33, :], 1.0, msq[:], Op.mult, Op.subtract
                )
                nc.scalar.activation(lnv[:], var[:], F.Ln, bias=epsc[0:1, 0:1])
                nc.scalar.activation(ru[0:1, 0:512], lnv[:], F.Exp, scale=-0.5)
                ln_state[t] = (mcp, ru)

            def ln_C2(t):
                mcp, ru = ln_state[t]
                # u = mean * rstd; then hi+lo bf16 split of rstd|u for the
                # broadcast matmuls (bf16 streams 1 col/cycle vs f32's 4,
                # hi+lo accumulation keeps ~fp32 precision).
                ruh = sm_pool.tile([1, 1024], dtb, tag="ruh")
                rul = sm_pool.tile([1, 1024], dtb, tag="rul")
                nc.vector.tensor_tensor(
                    ru[0:1, 512:1024], mcp[:], ru[0:1, 0:512], Op.mult
                )
                nc.vector.tensor_copy(ruh[:], ru[:])
                nc.vector.tensor_tensor(rul[:], ru[:], ruh[:], Op.subtract)
                ln_state[t] = (ruh, rul)

            def ln_D(t):
                sl = ts(t, 512)
                ruh, rul = ln_state.pop(t)
                bcr = pools["lnps"].tile([128, 512], dt, tag="ln")
                nc.tensor.matmul(
                    bcr[:], ones1[0:1, :], ruh[0:1, 0:512],
                    start=True, stop=False,
                )
                nc.tensor.matmul(
                    bcr[:], ones1[0:1, :], rul[0:1, 0:512],
                    start=False, stop=True,
                )
                bcu = pools["lnps"].tile([128, 512], dt, tag="ln")
                nc.tensor.matmul(
                    bcu[:], ones1[0:1, :], ruh[0:1, 512:1024],
                    start=True, stop=False,
                )
                nc.tensor.matmul(
                    bcu[:], ones1[0:1, :], rul[0:1, 512:1024],
                    start=False, stop=True,
                )
                tmp = gx_pool.tile([128, 512], dt, tag="xtmp")
                nc.vector.tensor_tensor(tmp[:], x_sb[:, sl], bcr[:], Op.mult)
                nc.vector.tensor_tensor(xn[:, sl], tmp[:], bcu[:], Op.subtract)
                if t < NIT:
                    qp = pools["lnps"].tile([128, 512], dt, tag="ln")
                    nc.tensor.matmul(qp[:], wq[:], xn[:, sl])
                    nc.vector.tensor_scalar(
                        q_stage[:, sl], qp[:], bq[:, 0:1], None, Op.add
                    )
                kp = pools["lnps"].tile([128, 512], dt, tag="ln")
                nc.tensor.matmul(kp[:], wk[:], xn[:, sl])
                nc.vector.tensor_copy(k_sb[:, sl], kp[:])

            def ln_E2(t):
                sl = ts(t, 512)
                for half in (0, 1):
                    vp = pools["lnps"].tile([128, 512], dt, tag="ln")
                    for cc in (0, 1):
                        jc = 4 * t + 2 * half + cc
                        nc.tensor.matmul(
                            vp[:, 128 * cc : 128 * cc + 128],
                            xn[:, ts(jc, 128)], wv[:],
                        )
                    base = (4 * t + 2 * half) * 512
                    dst = vaug[:, base : base + 1024].rearrange(
                        "p (c g e) -> p c g e", g=2, e=256
                    )
                    src = vp[:, 0:256].rearrange(
                        "p (c g e) -> p c g e", g=4, e=32
                    )
                    nc.vector.tensor_copy(
                        dst[:, :, :, 0:32], src[:, :, 0:2, :]
                    )
                    nc.vector.tensor_copy(
                        dst[:, :, :, 192:224], src[:, :, 2:4, :]
                    )

            def qpad_fill(t):
                sl = ts(t, 512)
                for h in range(HEADS):
                    nc.vector.tensor_copy(
                        q_pad[h][32 * h : 32 * h + 32, sl],
                        q_stage[32 * h : 32 * h + 32, sl],
                    )

            LN_FNS = (ln_A, ln_B, ln_C, ln_C2, ln_D, ln_E2)

            def ln_slot_schedule():
                """s-tiles 0-2 fully in the prologue (phase-major so the
                three tiles pipeline across engines); tiles 3-7 interleave
                into i-tile 0 COMPRESSED toward its start: the early exp
                stream is input-starved anyway, so the LN ACT/DVE work
                fills bubbles there and late i-tile 0 runs clean. Deadline:
                k/v/q(t) (slot E) before sims(4t), emitted at jc 4t-1."""
                sched = {}
                for fn, jc in zip(LN_FNS, (0, 1, 2, 4, 5, 6)):
                    sched.setdefault(jc, []).append((fn, 2))
                for t in range(3, 8):
                    e = min(2 * t + 6, 4 * t - 2)
                    offs = (2 * t - 6, 2 * t - 3, 2 * t, min(2 * t + 3, e - 3),
                            e - 1, e)
                    for fn, jc in zip(LN_FNS, offs):
                        sched.setdefault(jc, []).append((fn, t))
                for i in range(1, 4):
                    sched.setdefault(2 * i - 1, []).append((vaug_ones, i))
                # i-tile 1's q pads fill in i-tile 0's clean late zone
                sched.setdefault(20, []).append((qpad_fill, 1))
                return sched

            # ---------------- attention ----------------
            def make_tail(it, pairS, on_act):
                """Normalize+project closures run against the SBUF spill,
                sprinkled into the next i-tile's jc loop (off critical
                path). rec rows: 0 = even-head 1/denom, 32 = odd-head;
                cols 0:512 pairA, 512:1024 pairB. The final i-tile has
                nothing to hide under, so its reciprocals run as ACT
                ln/exp (~4x lower latency than DVE InstReciprocal)."""
                isl = ts(it, TI)
                st = {}

                def t_rec():
                    if on_act:
                        lnt = sm_pool.tile([1, 1024], dt, tag="lnt")
                        lnt2 = sm_pool.tile([1, 1024], dt, tag="lnt2")
                        nc.scalar.activation(lnt[:], pairS[32:33, :], F.Ln)
                        nc.scalar.activation(
                            rec[0:1, :], lnt[:], F.Exp, scale=-1.0
                        )
                        nc.scalar.activation(lnt2[:], pairS[96:97, :], F.Ln)
                        nc.scalar.activation(
                            rec[32:33, :], lnt2[:], F.Exp, scale=-1.0
                        )
                    else:
                        nc.vector.reciprocal(rec[0:1, :], pairS[32:33, :])
                        nc.vector.reciprocal(rec[32:33, :], pairS[96:97, :])

                def t_catA():
                    bcy = pools["duo"].tile([128, 1024], dt, tag="duo")
                    st["bc"] = bcy
                    st["yp"] = bcy
                    bc = bcy[0:97, 0:512]
                    nc.tensor.matmul(bc, ones2[0:33, :], rec[0:33, 0:512])
                    nc.vector.tensor_tensor(
                        catA[0:33, :], pairS[0:33, 0:512], bc[0:33, :], Op.mult
                    )
                    nc.vector.tensor_tensor(
                        catA[64:97, :], pairS[64:97, 0:512], bc[64:97, :],
                        Op.mult,
                    )

                def t_catB():
                    bc = st["bc"][0:97, 0:512]
                    nc.tensor.matmul(
                        st["yp"][:, 512:1024], woa[:, :], catA[0:97, :],
                        start=True, stop=False,
                    )
                    nc.tensor.matmul(bc, ones2[0:33, :], rec[0:33, 512:1024])
                    nc.vector.tensor_tensor(
                        catB[0:33, :], pairS[0:33, 512:1024], bc[0:33, :],
                        Op.mult,
                    )
                    nc.vector.tensor_tensor(
                        catB[64:97, :], pairS[64:97, 512:1024], bc[64:97, :],
                        Op.mult,
                    )

                def t_y():
                    yp = st["yp"][:, 512:1024]
                    nc.tensor.matmul(
                        yp, wob[:, :], catB[0:97, :], start=False, stop=True
                    )
                    ysb = ysb_pool.tile([128, TI], dt, tag="ysb")
                    nc.vector.tensor_scalar(
                        ysb[:], yp, bo[:, 0:1], None, Op.add
                    )
                    nc.sync.dma_start(y_d[:, isl], ysb[:])

                return [t_rec, t_catA, t_catB, t_y]

            def run_it(it, ln_sched, tail_pieces):
                isl = ts(it, TI)
                pairA = pair_pool.tile([128, TI], dt, tag="pair")
                pairB = pair_pool.tile([128, TI], dt, tag="pair")

                def emit_sims(jc):
                    ksl = ts(jc, JCHUNK)
                    duoX = pools["duo"].tile([128, 1024], dt, tag="duo")
                    nc.tensor.matmul(duoX[:, 0:512], k_sb[:, ksl], q_pad[0][:, isl])
                    nc.tensor.matmul(duoX[:, 512:1024], k_sb[:, ksl], q_pad[2][:, isl])
                    duoY = pools["duo"].tile([128, 1024], dt, tag="duo")
                    nc.tensor.matmul(duoY[:, 0:512], k_sb[:, ksl], q_pad[1][:, isl])
                    nc.tensor.matmul(duoY[:, 512:1024], k_sb[:, ksl], q_pad[3][:, isl])
                    return duoX, duoY

                duoX, duoY = emit_sims(0)
                for jc in range(NJC):
                    st, sp_ = jc == 0, jc == NJC - 1
                    vbase = jc * 512
                    expX = expo_pool.tile([128, 1024], dtb, tag="expo")
                    nc.scalar.activation(expX[:], duoX[:], F.Exp)
                    expY = expo_pool.tile([128, 1024], dtb, tag="expo")
                    nc.scalar.activation(expY[:], duoY[:], F.Exp)
                    if jc + 1 < NJC:
                        duoX, duoY = emit_sims(jc + 1)
                    nc.tensor.matmul(
                        pairA[:, :], vaug[:, vbase : vbase + 128],
                        expX[:, 0:512],
                        start=st, stop=False, skip_group_check=True,
                    )
                    nc.tensor.matmul(
                        pairA[:, :], vaug[:, vbase + 128 : vbase + 256],
                        expX[:, 512:1024],
                        start=False, stop=sp_, skip_group_check=True,
                    )
                    nc.tensor.matmul(
                        pairB[:, :], vaug[:, vbase + 256 : vbase + 384],
                        expY[:, 0:512],
                        start=st, stop=False, skip_group_check=True,
                    )
                    nc.tensor.matmul(
                        pairB[:, :], vaug[:, vbase + 384 : vbase + 512],
                        expY[:, 512:1024],
                        start=False, stop=sp_, skip_group_check=True,
                    )
                    # LN slots and tail pieces at the loop BOTTOM: this jc's
                    # avs were just emitted, so they don't (coarsely) wait
                    # on this slot's DVE work.
                    for fn, t in ln_sched.get(jc, ()):
                        fn(t)
                    # t_rec's ACT ln/exp ops ride the exp stream (~5us);
                    # the PE-visible bc/y pieces go late enough that the
                    # reciprocals are done when the PE queue reaches them.
                    if tail_pieces and jc in (1, 9, 11, 13, 15):
                        i = (1, 9, 11, 13, 15).index(jc)
                        if i < len(tail_pieces):
                            tail_pieces[i]()
                # spill pairs to SBUF: frees the pair banks after 2 quick
                # DVE copies; the tail runs later against the copy.
                pairS = spill_pool.tile([128, 1024], dt, tag="pairS")
                nc.vector.tensor_copy(pairS[:, 0:512], pairA[:])
                nc.vector.tensor_copy(pairS[:, 512:1024], pairB[:])
                return make_tail(it, pairS, on_act=True)

            sched = ln_slot_schedule()
            with (
                tc.tile_pool(name="lnps", bufs=2, space="PSUM") as lnp0,
                tc.tile_pool(name="duo0", bufs=2, space="PSUM") as duo0,
            ):
                pools["lnps"] = lnp0
                pools["duo"] = duo0
                vaug_ones(0)
                for fn in LN_FNS:
                    for t in range(2):
                        fn(t)
                qpad_fill(0)
                tail = run_it(0, sched, None)
            # it1-3: LN psum freed; a third duo buffer absorbs boundary
            # hiccups, and the tails borrow one slot for their bc|y.
            with tc.tile_pool(name="duo1", bufs=3, space="PSUM") as duo1:
                pools["duo"] = duo1
                for it in range(1, NIT):
                    if it + 1 < NIT:
                        tail.append(lambda t=it + 1: qpad_fill(t))
                    tail = run_it(it, {}, tail)
                for piece in tail:
                    piece()

    nc.compile()
    return nc


def _get_program():
    global _PROGRAM
    if _PROGRAM is None:
        _PROGRAM = _build_program()
    return _PROGRAM


def _prep_inputs(x, g, b, w_qkv, w_out, b_out):
    """Host-side sharding + weight folding. All tiny except x slicing."""
    f32 = np.float32
    x = np.asarray(x, f32).reshape(B, C, S)
    g_ = np.asarray(g, f32).reshape(C)
    b_ = np.asarray(b, f32).reshape(C)
    w_qkv = np.asarray(w_qkv, f32)
    w_out = np.asarray(w_out, f32)
    b_out = np.asarray(b_out, f32)

    import ml_dtypes

    bf16 = ml_dtypes.bfloat16
    scale = DIM_HEAD ** -0.5
    wg = w_qkv * g_[None, :]
    bias_qkv = w_qkv @ b_
    hid = HEADS * DIM_HEAD  # 128
    wq_t = np.ascontiguousarray((wg[0:hid] * scale).T).astype(bf16)
    wk_t = np.ascontiguousarray(wg[hid : 2 * hid].T).astype(bf16)
    wv_t = np.ascontiguousarray(wg[2 * hid : 3 * hid].T).astype(bf16)
    bias_q = np.ascontiguousarray((bias_qkv[0:hid] * scale).reshape(128, 1))
    # bias_k is dropped: it shifts all logits of a query equally and
    # softmax is shift-invariant (exact). bias_v folds exactly into the
    # output bias (attention rows sum to 1).
    bias_v = bias_qkv[2 * hid : 3 * hid]

    wo_t = w_out.T  # [hd, o]
    wo_a = np.zeros((97, 128), f32)
    wo_b = np.zeros((97, 128), f32)
    wo_a[0:32] = wo_t[0:32]     # head 0
    wo_a[64:96] = wo_t[64:96]   # head 2
    wo_b[0:32] = wo_t[32:64]    # head 1
    wo_b[64:96] = wo_t[96:128]  # head 3
    bias_o = np.ascontiguousarray((b_out + w_out @ bias_v).reshape(128, 1))

    shared = {
        "wq_t": wq_t,
        "wk_t": wk_t,
        "wv_t": wv_t,
        "wo_a": wo_a,
        "wo_b": wo_b,
        "bias_q": bias_q,
        "bias_o": bias_o,
        "zeros": np.zeros((128, 4096), bf16),
    }
    in_maps = []
    for core in range(N_CORES):
        bb, half = core // 2, core % 2
        if half == 0:
            xc = x[bb]
        else:
            xc = np.concatenate([x[bb][:, HALF:], x[bb][:, :HALF]], axis=1)
        m = {"x": np.ascontiguousarray(xc)}
        m.update(shared)
        in_maps.append(m)
    return in_maps


def _run(inputs, trace=False):
    from concourse.bass_utils import run_bass_kernel_spmd

    nc = _get_program()
    in_maps = _prep_inputs(**inputs)
    res = run_bass_kernel_spmd(
        nc, in_maps, core_ids=list(range(N_CORES)), trace=trace
    )
    y = np.empty((B, C, S), np.float32)
    for core in range(N_CORES):
        bb, half = core // 2, core % 2
        yc = res.results[core]["y"]
        if half == 0:
            y[bb][:, :HALF] = yc
        else:
            y[bb][:, HALF:] = yc
    return y.reshape(B, C, H, W), res


def kernel(x, g, b, w_qkv, w_out, b_out):
    out, _ = _run(
        {"x": x, "g": g, "b": b, "w_qkv": w_qkv, "w_out": w_out, "b_out": b_out}
    )
    return out


# revision 65
# speedup vs baseline: 1.1787x; 1.1787x over previous
"""Trainium2 Bass kernel for nn_Attention_10282151707309.

Reference computation:
  - channel LayerNorm over C=128 (biased var, eps=1e-5, affine g/b)
  - qkv = w_qkv @ xn (1x1 conv), 4 heads x 32 dims, q scaled by 1/sqrt(32)
  - full softmax attention over HW=4096 positions per (batch, head)
  - out = w_out @ attn_out + b_out

Sharding: 8 cores = (batch b in 0..3) x (spatial half in 0..1); each core
runs an identical program on its batch slice (spatially rolled so its own
2048 query columns are program-columns 0:2048 -- softmax is permutation-
equivariant over keys). No collectives; disjoint output slices.

Performance model (from perfetto traces of earlier versions):
  - The ACT engine is the wall: exp of 33.5M sim values at 128 lanes
    @1.2GHz, ~(N+390)/1.2 ns per N-element ACTIVATE. Steady state
    achieves one [128,1024] exp per ~1.2us; everything else must stay
    off ACT and off the exp stream's critical path.
  - PE streams full-mode (128,128)-tile bf16 matmuls at ~215-258ns per
    512 columns with LDWEIGHTS hidden; partial-tile matmuls are ~1.6x
    slower, so sim/av use only full [128,x] operands.
  - Engine queues are strictly in-order: any op whose producers are not
    long-finished head-of-line blocks its whole engine. All cross-engine
    chains (LN, tails) are therefore emission-scheduled several jc
    iterations after their producers.
Structure:
  - One activation-table preload (natural_log_exp_and_others) so Ln/Exp
    never swap tables (the original baseline lost 52us to 41 loads).
  - LN + projections are emission-interleaved under i-tile 0's attention
    in 5 slots per s-tile (xb/xsq -> stats -> rstd -> bc/xn -> proj),
    each ~2 jc after its producers. s-tiles 0-1 partially in prologue.
  - sim full-mode without a padded k: lhsT is the whole k_sb chunk (all
    4 heads' rows), rhs a per-head ZERO-PADDED q copy (zeros kill the
    other heads' k rows exactly). q pads and the zero-padded av lhsT
    tiles (vaug) are zero-filled by DMA from a host zeros tensor --
    no big on-chip memsets (gpsimd per-op overhead is ~1.2-2.2us).
  - k-bias dropped entirely (softmax shift-invariance, exact); v-bias
    folded into the output bias; g/b folded into the qkv weights.
  - i-tile tails are taken OFF the critical path: pairs are spilled
    PSUM->SBUF (2 DVE copies) at it end, which immediately frees the
    pair banks for the next i-tile; the normalize/project tail runs
    against the SBUF copy, sprinkled into the next i-tile's jc loop.
PSUM (8 banks): lnps [128,1024] (2; LN generations + tail bc|y)
  + duo 2x[128,1024] (4) + pairs 2x[128,512] (2).
"""

import numpy as np

HEADS = 4
DIM_HEAD = 32
B, C, H, W = 4, 128, 64, 64
S = H * W              # 4096 spatial positions
HALF = S // 2          # 2048 own query columns per core
TI = 512               # i-tile (query) size
NIT = HALF // TI       # 4 i-tiles
JCHUNK = 128           # j-chunk (key) size
NJC = S // JCHUNK      # 32 j-chunks
EPS = 1e-5
N_CORES = 8

_PROGRAM = None


def _build_program():
    """Build the (SPMD-identical) Bass program once per process."""
    import concourse.bass as bass  # noqa: F401
    import concourse.mybir as mybir
    import concourse.tile as tile
    from concourse import bacc
    from concourse.bass import ts

    dt = mybir.dt.float32
    dtr = mybir.dt.float32r
    dtb = mybir.dt.bfloat16
    F = mybir.ActivationFunctionType
    Op = mybir.AluOpType

    nc = bacc.Bacc(
        "TRN2",
        target_bir_lowering=False,
        debug=False,
        num_devices=N_CORES,
    )

    x_d = nc.dram_tensor("x", [C, S], dt, kind="ExternalInput").ap()
    wq_d = nc.dram_tensor("wq_t", [C, 128], dtb, kind="ExternalInput").ap()
    wk_d = nc.dram_tensor("wk_t", [C, 128], dtb, kind="ExternalInput").ap()
    wv_d = nc.dram_tensor("wv_t", [C, 128], dtb, kind="ExternalInput").ap()
    woa_d = nc.dram_tensor("wo_a", [97, 128], dt, kind="ExternalInput").ap()
    wob_d = nc.dram_tensor("wo_b", [97, 128], dt, kind="ExternalInput").ap()
    bq_d = nc.dram_tensor("bias_q", [128, 1], dt, kind="ExternalInput").ap()
    bo_d = nc.dram_tensor("bias_o", [128, 1], dt, kind="ExternalInput").ap()
    z_d = nc.dram_tensor("zeros", [128, 4096], dtb, kind="ExternalInput").ap()
    y_d = nc.dram_tensor("y", [C, HALF], dt, kind="ExternalOutput").ap()

    with tile.TileContext(nc) as tc:
        from contextlib import ExitStack

        with ExitStack() as ctx:
            const_pool = ctx.enter_context(tc.tile_pool(name="const", bufs=1))
            big_pool = ctx.enter_context(tc.tile_pool(name="big", bufs=1))

            # One table set (natural_log_exp_and_others, id 6) serves every
            # activation in this kernel (Exp, Ln); preload it once.
            nc.scalar.add_instruction(
                mybir.InstLoadActFuncSet(
                    name="act_preload", act_func_set_id=6, ins=[], outs=[]
                )
            )

            wq = const_pool.tile([C, 128], dtb, tag="wq")
            wk = const_pool.tile([C, 128], dtb, tag="wk")
            wv = const_pool.tile([C, 128], dtb, tag="wv")
            woa = const_pool.tile([97, 128], dt, tag="woa")
            wob = const_pool.tile([97, 128], dt, tag="wob")
            bq = const_pool.tile([128, 1], dt, tag="bq")
            bo = const_pool.tile([128, 1], dt, tag="bo")
            ones1 = const_pool.tile([1, 128], dtb, tag="ones1")
            onesC = const_pool.tile([128, 1], dtb, tag="onesC")
            # bc lhsT: row 0 -> out rows 0:33 (even-head reciprocal), row 32
            # -> out rows 64:97 (odd-head). Engine AP partition bases must be
            # 32-aligned, so the two reciprocal rows sit at partitions 0/32.
            ones2 = const_pool.tile([33, 97], dt, tag="ones2")
            epsc = const_pool.tile([1, 1], dt, tag="epsc")

            x_sb = big_pool.tile([C, S], dt, tag="x")
            xn = big_pool.tile([C, S], dtb, tag="xn")
            k_sb = big_pool.tile([128, S], dtb, tag="k")
            # vaug: per j-chunk four [128, 128] full-mode av lhsT tiles in
            # order [h0, h2, h1, h3]; h0/h1 carry (v^T | ones) at cols 0-32,
            # h2/h3 at cols 64-96, everything else zero.
            vaug = big_pool.tile([128, NJC * 512], dtb, tag="vaug")
            q_pad = [
                big_pool.tile(
                    [128, HALF], dtb, tag=f"qpad{h}", name=f"qpad{h}"
                )
                for h in range(HEADS)
            ]
            catA = big_pool.tile([128, TI], dt, tag="catA")
            catB = big_pool.tile([128, TI], dt, tag="catB")
            rec = big_pool.tile([64, 1024], dt, tag="rec")
            # biased q staging: one [128,512] PSUM-read op per q-tile here;
            # the four per-head pad writes become cheap bf16 SBUF copies
            # emitted in uncongested zones.
            q_stage = big_pool.tile([128, HALF], dtb, tag="qstage")

            # input DMAs; zero fills come from the host zeros tensor so no
            # engine spends time on them.
            # Each dma_start trigger costs ~0.6-0.7us on the serial Sync
            # queue, so the issue ORDER is the prologue critical path:
            # x tiles for the early LN chain first, zeros last.
            nc.sync.dma_start(x_sb[:, 0:512], x_d[:, 0:512])
            nc.sync.dma_start(wq[:], wq_d[:])
            nc.sync.dma_start(wk[:], wk_d[:])
            nc.sync.dma_start(x_sb[:, 512:1024], x_d[:, 512:1024])
            nc.sync.dma_start(wv[:], wv_d[:])
            nc.sync.dma_start(bq[:], bq_d[:])
            nc.sync.dma_start(x_sb[:, 1024:1536], x_d[:, 1024:1536])
            nc.sync.dma_start(woa[:], woa_d[:])
            nc.sync.dma_start(wob[:], wob_d[:])
            nc.sync.dma_start(bo[:], bo_d[:])
            nc.sync.dma_start(x_sb[:, 1536:2048], x_d[:, 1536:2048])
            nc.sync.dma_start(vaug[:, 0:4096], z_d[:, 0:4096])
            for h in range(HEADS):
                nc.sync.dma_start(q_pad[h][:], z_d[:, 0:HALF])
            for t in range(4, 8):
                nc.sync.dma_start(x_sb[:, ts(t, 512)], x_d[:, ts(t, 512)])
            for i in range(1, 4):
                nc.sync.dma_start(
                    vaug[:, 4096 * i : 4096 * (i + 1)], z_d[:, 0:4096]
                )

            nc.vector.memset(ones1[:], 1.0)
            nc.vector.memset(onesC[:], 1.0 / C)
            nc.vector.memset(ones2[:], 0.0)
            nc.vector.memset(ones2[0:1, 0:33], 1.0)
            nc.vector.memset(ones2[32:33, 64:97], 1.0)
            nc.vector.memset(rec[:], 0.0)
            nc.vector.memset(epsc[:], EPS)
            # cat rows 33:64 are read by the K=97 y matmul (against zero
            # rows of wo) -- zero once so they are never NaN. Row 32 is
            # rewritten by every tail (32-aligned memset base).
            nc.vector.memset(catA[32:64, :], 0.0)
            nc.vector.memset(catB[32:64, :], 0.0)

            def vaug_ones(i):
                """Softmax-denominator ones columns of vaug chunk i (8 jc):
                col 32 of the h0/h1 tiles, col 96 of h2/h3. Emitted per
                chunk so each waits only its own zero-DMA."""
                blk = vaug[:, 4096 * i : 4096 * (i + 1)].rearrange(
                    "p (c g e) -> p c g e", g=2, e=256
                )
                nc.vector.memset(blk[:, :, :, 32:33], 1.0)
                nc.vector.memset(blk[:, :, :, 224:225], 1.0)

            pair_pool = ctx.enter_context(
                tc.tile_pool(name="pair_ps", bufs=2, space="PSUM")
            )
            lnps_pool = ctx.enter_context(
                tc.tile_pool(name="lnps", bufs=2, space="PSUM")
            )
            duop = ctx.enter_context(
                tc.tile_pool(name="duo", bufs=2, space="PSUM")
            )
            expo_pool = ctx.enter_context(tc.tile_pool(name="expo", bufs=8))
            spill_pool = ctx.enter_context(tc.tile_pool(name="spill", bufs=2))
            ysb_pool = ctx.enter_context(tc.tile_pool(name="ysb", bufs=2))
            sm_pool = ctx.enter_context(tc.tile_pool(name="lnsm", bufs=2))
            gx_pool = ctx.enter_context(tc.tile_pool(name="lngx", bufs=3))

            # ---------------- LayerNorm + projections ----------------
            # Five emission slots per s-tile; in i-tile 0 they sit ~2 jc
            # after their producers so no engine head-of-line blocks.
            ln_state = {}

            def ln_A(t):
                sl = ts(t, 512)
                xb = gx_pool.tile([128, 512], dtb, tag="xb")
                xsq = gx_pool.tile([128, 512], dtb, tag="xsq")
                nc.vector.tensor_copy(xb[:], x_sb[:, sl])
                nc.vector.tensor_tensor(xsq[:], xb[:], xb[:], Op.mult)
                ln_state[t] = (xb, xsq)

            def ln_B(t):
                xb, xsq = ln_state[t]
                # sum at partition 0, sumsq at partition 32 of ONE 1-bank
                # generation (M=1 matmuls at col tile positions 0 / 32):
                # [128,512]x2buf ring halves the serialization depth of the
                # old [128,1024]x1buf ring.
                g = lnps_pool.tile([128, 512], dt, tag="ln")
                nc.tensor.matmul(g[0:1, :], onesC[:, 0:1], xb[:])
                nc.tensor.matmul(g[32:33, :], onesC[:, 0:1], xsq[:])
                ln_state[t] = g

            def ln_C(t):
                g = ln_state[t]
                mcp = sm_pool.tile([1, 512], dt, tag="mcp")
                msq = sm_pool.tile([1, 512], dt, tag="msq")
                var = sm_pool.tile([1, 512], dt, tag="var")
                lnv = sm_pool.tile([1, 512], dt, tag="lnv")
                ru = sm_pool.tile([1, 1024], dt, tag="ru")
                nc.vector.tensor_copy(mcp[:], g[0:1, :])
                nc.vector.tensor_tensor(msq[:], mcp[:], mcp[:], Op.mult)
                nc.vector.scalar_tensor_tensor(
                    var[:], g[32:33, :], 1.0, msq[:], Op.mult, Op.subtract
                )
                nc.scalar.activation(lnv[:], var[:], F.Ln, bias=epsc[0:1, 0:1])
                nc.scalar.activation(ru[0:1, 0:512], lnv[:], F.Exp, scale=-0.5)
                ln_state[t] = (mcp, ru)

            def ln_C2(t):
                mcp, ru = ln_state[t]
                # u = mean * rstd; then hi+lo bf16 split of rstd|u for the
                # broadcast matmuls (bf16 streams 1 col/cycle vs f32's 4,
                # hi+lo accumulation keeps ~fp32 precision).
                ruh = sm_pool.tile([1, 1024], dtb, tag="ruh")
                rul = sm_pool.tile([1, 1024], dtb, tag="rul")
                nc.vector.tensor_tensor(
                    ru[0:1, 512:1024], mcp[:], ru[0:1, 0:512], Op.mult
                )
                nc.vector.tensor_copy(ruh[:], ru[:])
                nc.vector.tensor_tensor(rul[:], ru[:], ruh[:], Op.subtract)
                ln_state[t] = (ruh, rul)

            def ln_D(t):
                sl = ts(t, 512)
                ruh, rul = ln_state.pop(t)
                bcr = lnps_pool.tile([128, 512], dt, tag="ln")
                nc.tensor.matmul(
                    bcr[:], ones1[0:1, :], ruh[0:1, 0:512],
                    start=True, stop=False,
                )
                nc.tensor.matmul(
                    bcr[:], ones1[0:1, :], rul[0:1, 0:512],
                    start=False, stop=True,
                )
                bcu = lnps_pool.tile([128, 512], dt, tag="ln")
                nc.tensor.matmul(
                    bcu[:], ones1[0:1, :], ruh[0:1, 512:1024],
                    start=True, stop=False,
                )
                nc.tensor.matmul(
                    bcu[:], ones1[0:1, :], rul[0:1, 512:1024],
                    start=False, stop=True,
                )
                tmp = gx_pool.tile([128, 512], dt, tag="xtmp")
                nc.vector.tensor_tensor(tmp[:], x_sb[:, sl], bcr[:], Op.mult)
                nc.vector.tensor_tensor(xn[:, sl], tmp[:], bcu[:], Op.subtract)
                if t < NIT:
                    qp = lnps_pool.tile([128, 512], dt, tag="ln")
                    nc.tensor.matmul(qp[:], wq[:], xn[:, sl])
                    nc.vector.tensor_scalar(
                        q_stage[:, sl], qp[:], bq[:, 0:1], None, Op.add
                    )
                kp = lnps_pool.tile([128, 512], dt, tag="ln")
                nc.tensor.matmul(kp[:], wk[:], xn[:, sl])
                nc.vector.tensor_copy(k_sb[:, sl], kp[:])

            def ln_E2(t):
                sl = ts(t, 512)
                for half in (0, 1):
                    vp = lnps_pool.tile([128, 512], dt, tag="ln")
                    for cc in (0, 1):
                        jc = 4 * t + 2 * half + cc
                        nc.tensor.matmul(
                            vp[:, 128 * cc : 128 * cc + 128],
                            xn[:, ts(jc, 128)], wv[:],
                        )
                    base = (4 * t + 2 * half) * 512
                    dst = vaug[:, base : base + 1024].rearrange(
                        "p (c g e) -> p c g e", g=2, e=256
                    )
                    src = vp[:, 0:256].rearrange(
                        "p (c g e) -> p c g e", g=4, e=32
                    )
                    nc.vector.tensor_copy(
                        dst[:, :, :, 0:32], src[:, :, 0:2, :]
                    )
                    nc.vector.tensor_copy(
                        dst[:, :, :, 192:224], src[:, :, 2:4, :]
                    )

            def qpad_fill(t):
                sl = ts(t, 512)
                for h in range(HEADS):
                    nc.vector.tensor_copy(
                        q_pad[h][32 * h : 32 * h + 32, sl],
                        q_stage[32 * h : 32 * h + 32, sl],
                    )

            LN_FNS = (ln_A, ln_B, ln_C, ln_C2, ln_D, ln_E2)

            def ln_slot_schedule():
                """s-tiles 0-2 fully in the prologue (phase-major so the
                three tiles pipeline across engines); tiles 3-7 interleave
                into i-tile 0 COMPRESSED toward its start: the early exp
                stream is input-starved anyway, so the LN ACT/DVE work
                fills bubbles there and late i-tile 0 runs clean. Deadline:
                k/v/q(t) (slot E) before sims(4t), emitted at jc 4t-1."""
                sched = {}
                for fn, jc in zip(LN_FNS, (0, 1, 2, 4, 5, 6)):
                    sched.setdefault(jc, []).append((fn, 2))
                for t in range(3, 8):
                    e = min(2 * t + 6, 4 * t - 2)
                    offs = (2 * t - 6, 2 * t - 3, 2 * t, min(2 * t + 3, e - 3),
                            e - 1, e)
                    for fn, jc in zip(LN_FNS, offs):
                        sched.setdefault(jc, []).append((fn, t))
                for i in range(1, 4):
                    sched.setdefault(2 * i - 1, []).append((vaug_ones, i))
                # i-tile 1's q pads fill in i-tile 0's clean late zone
                sched.setdefault(20, []).append((qpad_fill, 1))
                return sched

            # ---------------- attention ----------------
            def make_tail(it, pairS, on_act):
                """Normalize+project closures run against the SBUF spill,
                sprinkled into the next i-tile's jc loop (off critical
                path). rec rows: 0 = even-head 1/denom, 32 = odd-head;
                cols 0:512 pairA, 512:1024 pairB. The final i-tile has
                nothing to hide under, so its reciprocals run as ACT
                ln/exp (~4x lower latency than DVE InstReciprocal)."""
                isl = ts(it, TI)
                st = {}

                def t_rec():
                    if on_act:
                        lnt = sm_pool.tile([1, 1024], dt, tag="lnt")
                        lnt2 = sm_pool.tile([1, 1024], dt, tag="lnt2")
                        nc.scalar.activation(lnt[:], pairS[32:33, :], F.Ln)
                        nc.scalar.activation(
                            rec[0:1, :], lnt[:], F.Exp, scale=-1.0
                        )
                        nc.scalar.activation(lnt2[:], pairS[96:97, :], F.Ln)
                        nc.scalar.activation(
                            rec[32:33, :], lnt2[:], F.Exp, scale=-1.0
                        )
                    else:
                        nc.vector.reciprocal(rec[0:1, :], pairS[32:33, :])
                        nc.vector.reciprocal(rec[32:33, :], pairS[96:97, :])

                def t_catA():
                    bct = lnps_pool.tile([128, 512], dt, tag="ln")
                    ypt = lnps_pool.tile([128, 512], dt, tag="ln")
                    st["bc"] = bct
                    st["yp"] = ypt
                    bc = bct[0:97, :]
                    nc.tensor.matmul(bc, ones2[0:33, :], rec[0:33, 0:512])
                    nc.vector.tensor_tensor(
                        catA[0:33, :], pairS[0:33, 0:512], bc[0:33, :], Op.mult
                    )
                    nc.vector.tensor_tensor(
                        catA[64:97, :], pairS[64:97, 0:512], bc[64:97, :],
                        Op.mult,
                    )

                def t_catB():
                    bc = st["bc"][0:97, :]
                    nc.tensor.matmul(
                        st["yp"][:, :], woa[:, :], catA[0:97, :],
                        start=True, stop=False,
                    )
                    nc.tensor.matmul(bc, ones2[0:33, :], rec[0:33, 512:1024])
                    nc.vector.tensor_tensor(
                        catB[0:33, :], pairS[0:33, 512:1024], bc[0:33, :],
                        Op.mult,
                    )
                    nc.vector.tensor_tensor(
                        catB[64:97, :], pairS[64:97, 512:1024], bc[64:97, :],
                        Op.mult,
                    )

                def t_y():
                    yp = st["yp"][:, :]
                    nc.tensor.matmul(
                        yp, wob[:, :], catB[0:97, :], start=False, stop=True
                    )
                    ysb = ysb_pool.tile([128, TI], dt, tag="ysb")
                    nc.vector.tensor_scalar(
                        ysb[:], yp, bo[:, 0:1], None, Op.add
                    )
                    nc.sync.dma_start(y_d[:, isl], ysb[:])

                return [t_rec, t_catA, t_catB, t_y]

            def run_it(it, ln_sched, tail_pieces):
                isl = ts(it, TI)
                pairA = pair_pool.tile([128, TI], dt, tag="pair")
                pairB = pair_pool.tile([128, TI], dt, tag="pair")

                def emit_sims(jc):
                    ksl = ts(jc, JCHUNK)
                    duoX = duop.tile([128, 1024], dt, tag="duo")
                    nc.tensor.matmul(duoX[:, 0:512], k_sb[:, ksl], q_pad[0][:, isl])
                    nc.tensor.matmul(duoX[:, 512:1024], k_sb[:, ksl], q_pad[2][:, isl])
                    duoY = duop.tile([128, 1024], dt, tag="duo")
                    nc.tensor.matmul(duoY[:, 0:512], k_sb[:, ksl], q_pad[1][:, isl])
                    nc.tensor.matmul(duoY[:, 512:1024], k_sb[:, ksl], q_pad[3][:, isl])
                    return duoX, duoY

                duoX, duoY = emit_sims(0)
                for jc in range(NJC):
                    st, sp_ = jc == 0, jc == NJC - 1
                    vbase = jc * 512
                    expX = expo_pool.tile([128, 1024], dtb, tag="expo")
                    nc.scalar.activation(expX[:], duoX[:], F.Exp)
                    expY = expo_pool.tile([128, 1024], dtb, tag="expo")
                    nc.scalar.activation(expY[:], duoY[:], F.Exp)
                    if jc + 1 < NJC:
                        duoX, duoY = emit_sims(jc + 1)
                    nc.tensor.matmul(
                        pairA[:, :], vaug[:, vbase : vbase + 128],
                        expX[:, 0:512],
                        start=st, stop=False, skip_group_check=True,
                    )
                    nc.tensor.matmul(
                        pairA[:, :], vaug[:, vbase + 128 : vbase + 256],
                        expX[:, 512:1024],
                        start=False, stop=sp_, skip_group_check=True,
                    )
                    nc.tensor.matmul(
                        pairB[:, :], vaug[:, vbase + 256 : vbase + 384],
                        expY[:, 0:512],
                        start=st, stop=False, skip_group_check=True,
                    )
                    nc.tensor.matmul(
                        pairB[:, :], vaug[:, vbase + 384 : vbase + 512],
                        expY[:, 512:1024],
                        start=False, stop=sp_, skip_group_check=True,
                    )
                    # LN slots and tail pieces at the loop BOTTOM: this jc's
                    # avs were just emitted, so they don't (coarsely) wait
                    # on this slot's DVE work.
                    for fn, t in ln_sched.get(jc, ()):
                        fn(t)
                    # t_rec's ACT ln/exp ops ride the exp stream (~5us);
                    # the PE-visible bc/y pieces go late enough that the
                    # reciprocals are done when the PE queue reaches them.
                    if tail_pieces and jc in (1, 9, 11, 13, 15):
                        i = (1, 9, 11, 13, 15).index(jc)
                        if i < len(tail_pieces):
                            tail_pieces[i]()
                # spill pairs to SBUF: frees the pair banks after 2 quick
                # DVE copies; the tail runs later against the copy.
                pairS = spill_pool.tile([128, 1024], dt, tag="pairS")
                nc.vector.tensor_copy(pairS[:, 0:512], pairA[:])
                nc.vector.tensor_copy(pairS[:, 512:1024], pairB[:])
                return make_tail(it, pairS, on_act=True)

            sched = ln_slot_schedule()
            vaug_ones(0)
            for fn in LN_FNS:
                for t in range(2):
                    fn(t)
            qpad_fill(0)
            tail = run_it(0, sched, None)
            for it in range(1, NIT):
                if it + 1 < NIT:
                    tail.append(lambda t=it + 1: qpad_fill(t))
                tail = run_it(it, {}, tail)
            for piece in tail:
                piece()

    nc.compile()
    return nc


def _get_program():
    global _PROGRAM
    if _PROGRAM is None:
        _PROGRAM = _build_program()
    return _PROGRAM


def _prep_inputs(x, g, b, w_qkv, w_out, b_out):
    """Host-side sharding + weight folding. All tiny except x slicing."""
    f32 = np.float32
    x = np.asarray(x, f32).reshape(B, C, S)
    g_ = np.asarray(g, f32).reshape(C)
    b_ = np.asarray(b, f32).reshape(C)
    w_qkv = np.asarray(w_qkv, f32)
    w_out = np.asarray(w_out, f32)
    b_out = np.asarray(b_out, f32)

    import ml_dtypes

    bf16 = ml_dtypes.bfloat16
    scale = DIM_HEAD ** -0.5
    wg = w_qkv * g_[None, :]
    bias_qkv = w_qkv @ b_
    hid = HEADS * DIM_HEAD  # 128
    wq_t = np.ascontiguousarray((wg[0:hid] * scale).T).astype(bf16)
    wk_t = np.ascontiguousarray(wg[hid : 2 * hid].T).astype(bf16)
    wv_t = np.ascontiguousarray(wg[2 * hid : 3 * hid].T).astype(bf16)
    bias_q = np.ascontiguousarray((bias_qkv[0:hid] * scale).reshape(128, 1))
    # bias_k is dropped: it shifts all logits of a query equally and
    # softmax is shift-invariant (exact). bias_v folds exactly into the
    # output bias (attention rows sum to 1).
    bias_v = bias_qkv[2 * hid : 3 * hid]

    wo_t = w_out.T  # [hd, o]
    wo_a = np.zeros((97, 128), f32)
    wo_b = np.zeros((97, 128), f32)
    wo_a[0:32] = wo_t[0:32]     # head 0
    wo_a[64:96] = wo_t[64:96]   # head 2
    wo_b[0:32] = wo_t[32:64]    # head 1
    wo_b[64:96] = wo_t[96:128]  # head 3
    bias_o = np.ascontiguousarray((b_out + w_out @ bias_v).reshape(128, 1))

    shared = {
        "wq_t": wq_t,
        "wk_t": wk_t,
        "wv_t": wv_t,
        "wo_a": wo_a,
        "wo_b": wo_b,
        "bias_q": bias_q,
        "bias_o": bias_o,
        "zeros": np.zeros((128, 4096), bf16),
    }
    in_maps = []
    for core in range(N_CORES):
        bb, half = core // 2, core % 2
        if half == 0:
            xc = x[bb]
        else:
            xc = np.concatenate([x[bb][:, HALF:], x[bb][:, :HALF]], axis=1)
        m = {"x": np.ascontiguousarray(xc)}
        m.update(shared)
        in_maps.append(m)
    return in_maps


def _run(inputs, trace=False):
    from concourse.bass_utils import run_bass_kernel_spmd

    nc = _get_program()
    in_maps = _prep_inputs(**inputs)
    res = run_bass_kernel_spmd(
        nc, in_maps, core_ids=list(range(N_CORES)), trace=trace
    )
    y = np.empty((B, C, S), np.float32)
    for core in range(N_CORES):
        bb, half = core // 2, core % 2
        yc = res.results[core]["y"]
        if half == 0:
            y[bb][:, :HALF] = yc
        else:
            y[bb][:, HALF:] = yc
    return y.reshape(B, C, H, W), res


def kernel(x, g, b, w_qkv, w_out, b_out):
    out, _ = _run(
        {"x": x, "g": g, "b": b, "w_qkv": w_qkv, "w_out": w_out, "b_out": b_out}
    )
    return out


# revision 66
# speedup vs baseline: 1.1844x; 1.0048x over previous
"""Trainium2 Bass kernel for nn_Attention_10282151707309.

Reference computation:
  - channel LayerNorm over C=128 (biased var, eps=1e-5, affine g/b)
  - qkv = w_qkv @ xn (1x1 conv), 4 heads x 32 dims, q scaled by 1/sqrt(32)
  - full softmax attention over HW=4096 positions per (batch, head)
  - out = w_out @ attn_out + b_out

Sharding: 8 cores = (batch b in 0..3) x (spatial half in 0..1); each core
runs an identical program on its batch slice (spatially rolled so its own
2048 query columns are program-columns 0:2048 -- softmax is permutation-
equivariant over keys). No collectives; disjoint output slices.

Performance model (from perfetto traces of earlier versions):
  - The ACT engine is the wall: exp of 33.5M sim values at 128 lanes
    @1.2GHz, ~(N+390)/1.2 ns per N-element ACTIVATE. Steady state
    achieves one [128,1024] exp per ~1.2us; everything else must stay
    off ACT and off the exp stream's critical path.
  - PE streams full-mode (128,128)-tile bf16 matmuls at ~215-258ns per
    512 columns with LDWEIGHTS hidden; partial-tile matmuls are ~1.6x
    slower, so sim/av use only full [128,x] operands.
  - Engine queues are strictly in-order: any op whose producers are not
    long-finished head-of-line blocks its whole engine. All cross-engine
    chains (LN, tails) are therefore emission-scheduled several jc
    iterations after their producers.
Structure:
  - One activation-table preload (natural_log_exp_and_others) so Ln/Exp
    never swap tables (the original baseline lost 52us to 41 loads).
  - LN + projections are emission-interleaved under i-tile 0's attention
    in 5 slots per s-tile (xb/xsq -> stats -> rstd -> bc/xn -> proj),
    each ~2 jc after its producers. s-tiles 0-1 partially in prologue.
  - sim full-mode without a padded k: lhsT is the whole k_sb chunk (all
    4 heads' rows), rhs a per-head ZERO-PADDED q copy (zeros kill the
    other heads' k rows exactly). q pads and the zero-padded av lhsT
    tiles (vaug) are zero-filled by DMA from a host zeros tensor --
    no big on-chip memsets (gpsimd per-op overhead is ~1.2-2.2us).
  - k-bias dropped entirely (softmax shift-invariance, exact); v-bias
    folded into the output bias; g/b folded into the qkv weights.
  - i-tile tails are taken OFF the critical path: pairs are spilled
    PSUM->SBUF (2 DVE copies) at it end, which immediately frees the
    pair banks for the next i-tile; the normalize/project tail runs
    against the SBUF copy, sprinkled into the next i-tile's jc loop.
PSUM (8 banks): lnps [128,1024] (2; LN generations + tail bc|y)
  + duo 2x[128,1024] (4) + pairs 2x[128,512] (2).
"""

import numpy as np

HEADS = 4
DIM_HEAD = 32
B, C, H, W = 4, 128, 64, 64
S = H * W              # 4096 spatial positions
HALF = S // 2          # 2048 own query columns per core
TI = 512               # i-tile (query) size
NIT = HALF // TI       # 4 i-tiles
JCHUNK = 128           # j-chunk (key) size
NJC = S // JCHUNK      # 32 j-chunks
EPS = 1e-5
N_CORES = 8

_PROGRAM = None


def _build_program():
    """Build the (SPMD-identical) Bass program once per process."""
    import concourse.bass as bass  # noqa: F401
    import concourse.mybir as mybir
    import concourse.tile as tile
    from concourse import bacc
    from concourse.bass import ts

    dt = mybir.dt.float32
    dtr = mybir.dt.float32r
    dtb = mybir.dt.bfloat16
    F = mybir.ActivationFunctionType
    Op = mybir.AluOpType

    nc = bacc.Bacc(
        "TRN2",
        target_bir_lowering=False,
        debug=False,
        num_devices=N_CORES,
    )

    x_d = nc.dram_tensor("x", [C, S], dt, kind="ExternalInput").ap()
    wq_d = nc.dram_tensor("wq_t", [C, 128], dtb, kind="ExternalInput").ap()
    wk_d = nc.dram_tensor("wk_t", [C, 128], dtb, kind="ExternalInput").ap()
    wv_d = nc.dram_tensor("wv_t", [C, 128], dtb, kind="ExternalInput").ap()
    woa_d = nc.dram_tensor("wo_a", [97, 128], dt, kind="ExternalInput").ap()
    wob_d = nc.dram_tensor("wo_b", [97, 128], dt, kind="ExternalInput").ap()
    bq_d = nc.dram_tensor("bias_q", [128, 1], dt, kind="ExternalInput").ap()
    bo_d = nc.dram_tensor("bias_o", [128, 1], dt, kind="ExternalInput").ap()
    z_d = nc.dram_tensor("zeros", [128, 4096], dtb, kind="ExternalInput").ap()
    y_d = nc.dram_tensor("y", [C, HALF], dt, kind="ExternalOutput").ap()

    with tile.TileContext(nc) as tc:
        from contextlib import ExitStack

        with ExitStack() as ctx:
            const_pool = ctx.enter_context(tc.tile_pool(name="const", bufs=1))
            big_pool = ctx.enter_context(tc.tile_pool(name="big", bufs=1))

            # One table set (natural_log_exp_and_others, id 6) serves every
            # activation in this kernel (Exp, Ln); preload it once.
            nc.scalar.add_instruction(
                mybir.InstLoadActFuncSet(
                    name="act_preload", act_func_set_id=6, ins=[], outs=[]
                )
            )

            wq = const_pool.tile([C, 128], dtb, tag="wq")
            wk = const_pool.tile([C, 128], dtb, tag="wk")
            wv = const_pool.tile([C, 128], dtb, tag="wv")
            woa = const_pool.tile([97, 128], dt, tag="woa")
            wob = const_pool.tile([97, 128], dt, tag="wob")
            bq = const_pool.tile([128, 1], dt, tag="bq")
            bo = const_pool.tile([128, 1], dt, tag="bo")
            ones1 = const_pool.tile([1, 128], dtb, tag="ones1")
            onesC = const_pool.tile([128, 1], dtb, tag="onesC")
            # bc lhsT: row 0 -> out rows 0:33 (even-head reciprocal), row 32
            # -> out rows 64:97 (odd-head). Engine AP partition bases must be
            # 32-aligned, so the two reciprocal rows sit at partitions 0/32.
            ones2 = const_pool.tile([33, 97], dt, tag="ones2")
            epsc = const_pool.tile([1, 1], dt, tag="epsc")

            x_sb = big_pool.tile([C, S], dt, tag="x")
            xn = big_pool.tile([C, S], dtb, tag="xn")
            k_sb = big_pool.tile([128, S], dtb, tag="k")
            # vaug: per j-chunk four [128, 128] full-mode av lhsT tiles in
            # order [h0, h2, h1, h3]; h0/h1 carry (v^T | ones) at cols 0-32,
            # h2/h3 at cols 64-96, everything else zero.
            vaug = big_pool.tile([128, NJC * 512], dtb, tag="vaug")
            q_pad = [
                big_pool.tile(
                    [128, HALF], dtb, tag=f"qpad{h}", name=f"qpad{h}"
                )
                for h in range(HEADS)
            ]
            catA = big_pool.tile([128, TI], dt, tag="catA")
            catB = big_pool.tile([128, TI], dt, tag="catB")
            rec = big_pool.tile([64, 1024], dt, tag="rec")
            # biased q staging: one [128,512] PSUM-read op per q-tile here;
            # the four per-head pad writes become cheap bf16 SBUF copies
            # emitted in uncongested zones.
            q_stage = big_pool.tile([128, HALF], dtb, tag="qstage")

            # input DMAs; zero fills come from the host zeros tensor so no
            # engine spends time on them.
            # Each dma_start trigger costs ~0.6-0.7us on the serial Sync
            # queue, so the issue ORDER is the prologue critical path:
            # x tiles for the early LN chain first, zeros last.
            nc.sync.dma_start(x_sb[:, 0:512], x_d[:, 0:512])
            nc.sync.dma_start(wq[:], wq_d[:])
            nc.sync.dma_start(wk[:], wk_d[:])
            nc.sync.dma_start(x_sb[:, 512:1024], x_d[:, 512:1024])
            nc.sync.dma_start(wv[:], wv_d[:])
            nc.sync.dma_start(bq[:], bq_d[:])
            nc.sync.dma_start(x_sb[:, 1024:1536], x_d[:, 1024:1536])
            nc.sync.dma_start(woa[:], woa_d[:])
            nc.sync.dma_start(wob[:], wob_d[:])
            nc.sync.dma_start(bo[:], bo_d[:])
            nc.sync.dma_start(x_sb[:, 1536:2048], x_d[:, 1536:2048])
            nc.sync.dma_start(vaug[:, 0:4096], z_d[:, 0:4096])
            for h in range(HEADS):
                nc.sync.dma_start(q_pad[h][:], z_d[:, 0:HALF])
            for t in range(4, 8):
                nc.sync.dma_start(x_sb[:, ts(t, 512)], x_d[:, ts(t, 512)])
            for i in range(1, 4):
                nc.sync.dma_start(
                    vaug[:, 4096 * i : 4096 * (i + 1)], z_d[:, 0:4096]
                )

            nc.vector.memset(ones1[:], 1.0)
            nc.vector.memset(onesC[:], 1.0 / C)
            nc.vector.memset(ones2[:], 0.0)
            nc.vector.memset(ones2[0:1, 0:33], 1.0)
            nc.vector.memset(ones2[32:33, 64:97], 1.0)
            nc.vector.memset(rec[:], 0.0)
            nc.vector.memset(epsc[:], EPS)
            # cat rows 33:64 are read by the K=97 y matmul (against zero
            # rows of wo) -- zero once so they are never NaN. Row 32 is
            # rewritten by every tail (32-aligned memset base).
            nc.vector.memset(catA[32:64, :], 0.0)
            nc.vector.memset(catB[32:64, :], 0.0)

            def vaug_ones(i):
                """Softmax-denominator ones columns of vaug chunk i (8 jc):
                col 32 of the h0/h1 tiles, col 96 of h2/h3. Emitted per
                chunk so each waits only its own zero-DMA."""
                blk = vaug[:, 4096 * i : 4096 * (i + 1)].rearrange(
                    "p (c g e) -> p c g e", g=2, e=256
                )
                nc.vector.memset(blk[:, :, :, 32:33], 1.0)
                nc.vector.memset(blk[:, :, :, 224:225], 1.0)

            pair_pool = ctx.enter_context(
                tc.tile_pool(name="pair_ps", bufs=2, space="PSUM")
            )
            pools = {}
            expo_pool = ctx.enter_context(tc.tile_pool(name="expo", bufs=8))
            spill_pool = ctx.enter_context(tc.tile_pool(name="spill", bufs=2))
            ysb_pool = ctx.enter_context(tc.tile_pool(name="ysb", bufs=2))
            sm_pool = ctx.enter_context(tc.tile_pool(name="lnsm", bufs=2))
            gx_pool = ctx.enter_context(tc.tile_pool(name="lngx", bufs=3))

            # ---------------- LayerNorm + projections ----------------
            # Five emission slots per s-tile; in i-tile 0 they sit ~2 jc
            # after their producers so no engine head-of-line blocks.
            ln_state = {}

            def ln_A(t):
                sl = ts(t, 512)
                xb = gx_pool.tile([128, 512], dtb, tag="xb")
                xsq = gx_pool.tile([128, 512], dtb, tag="xsq")
                nc.vector.tensor_copy(xb[:], x_sb[:, sl])
                nc.vector.tensor_tensor(xsq[:], xb[:], xb[:], Op.mult)
                ln_state[t] = (xb, xsq)

            def ln_B(t):
                xb, xsq = ln_state[t]
                # sum at partition 0, sumsq at partition 32 of ONE 1-bank
                # generation (M=1 matmuls at col tile positions 0 / 32):
                # [128,512]x2buf ring halves the serialization depth of the
                # old [128,1024]x1buf ring.
                g = pools["lnps"].tile([128, 512], dt, tag="ln")
                nc.tensor.matmul(g[0:1, :], onesC[:, 0:1], xb[:])
                nc.tensor.matmul(g[32:33, :], onesC[:, 0:1], xsq[:])
                ln_state[t] = g

            def ln_C(t):
                g = ln_state[t]
                mcp = sm_pool.tile([1, 512], dt, tag="mcp")
                msq = sm_pool.tile([1, 512], dt, tag="msq")
                var = sm_pool.tile([1, 512], dt, tag="var")
                lnv = sm_pool.tile([1, 512], dt, tag="lnv")
                ru = sm_pool.tile([1, 1024], dt, tag="ru")
                nc.vector.tensor_copy(mcp[:], g[0:1, :])
                nc.vector.tensor_tensor(msq[:], mcp[:], mcp[:], Op.mult)
                nc.vector.scalar_tensor_tensor(
                    var[:], g[32:33, :], 1.0, msq[:], Op.mult, Op.subtract
                )
                nc.scalar.activation(lnv[:], var[:], F.Ln, bias=epsc[0:1, 0:1])
                nc.scalar.activation(ru[0:1, 0:512], lnv[:], F.Exp, scale=-0.5)
                ln_state[t] = (mcp, ru)

            def ln_C2(t):
                mcp, ru = ln_state[t]
                # u = mean * rstd; then hi+lo bf16 split of rstd|u for the
                # broadcast matmuls (bf16 streams 1 col/cycle vs f32's 4,
                # hi+lo accumulation keeps ~fp32 precision).
                ruh = sm_pool.tile([1, 1024], dtb, tag="ruh")
                rul = sm_pool.tile([1, 1024], dtb, tag="rul")
                nc.vector.tensor_tensor(
                    ru[0:1, 512:1024], mcp[:], ru[0:1, 0:512], Op.mult
                )
                nc.vector.tensor_copy(ruh[:], ru[:])
                nc.vector.tensor_tensor(rul[:], ru[:], ruh[:], Op.subtract)
                ln_state[t] = (ruh, rul)

            def ln_D(t):
                sl = ts(t, 512)
                ruh, rul = ln_state.pop(t)
                bcr = pools["lnps"].tile([128, 512], dt, tag="ln")
                nc.tensor.matmul(
                    bcr[:], ones1[0:1, :], ruh[0:1, 0:512],
                    start=True, stop=False,
                )
                nc.tensor.matmul(
                    bcr[:], ones1[0:1, :], rul[0:1, 0:512],
                    start=False, stop=True,
                )
                bcu = pools["lnps"].tile([128, 512], dt, tag="ln")
                nc.tensor.matmul(
                    bcu[:], ones1[0:1, :], ruh[0:1, 512:1024],
                    start=True, stop=False,
                )
                nc.tensor.matmul(
                    bcu[:], ones1[0:1, :], rul[0:1, 512:1024],
                    start=False, stop=True,
                )
                tmp = gx_pool.tile([128, 512], dt, tag="xtmp")
                nc.vector.tensor_tensor(tmp[:], x_sb[:, sl], bcr[:], Op.mult)
                nc.vector.tensor_tensor(xn[:, sl], tmp[:], bcu[:], Op.subtract)
                if t < NIT:
                    qp = pools["lnps"].tile([128, 512], dt, tag="ln")
                    nc.tensor.matmul(qp[:], wq[:], xn[:, sl])
                    nc.vector.tensor_scalar(
                        q_stage[:, sl], qp[:], bq[:, 0:1], None, Op.add
                    )
                kp = pools["lnps"].tile([128, 512], dt, tag="ln")
                nc.tensor.matmul(kp[:], wk[:], xn[:, sl])
                nc.vector.tensor_copy(k_sb[:, sl], kp[:])

            def ln_E2(t):
                sl = ts(t, 512)
                for half in (0, 1):
                    vp = pools["lnps"].tile([128, 512], dt, tag="ln")
                    for cc in (0, 1):
                        jc = 4 * t + 2 * half + cc
                        nc.tensor.matmul(
                            vp[:, 128 * cc : 128 * cc + 128],
                            xn[:, ts(jc, 128)], wv[:],
                        )
                    base = (4 * t + 2 * half) * 512
                    dst = vaug[:, base : base + 1024].rearrange(
                        "p (c g e) -> p c g e", g=2, e=256
                    )
                    src = vp[:, 0:256].rearrange(
                        "p (c g e) -> p c g e", g=4, e=32
                    )
                    nc.vector.tensor_copy(
                        dst[:, :, :, 0:32], src[:, :, 0:2, :]
                    )
                    nc.vector.tensor_copy(
                        dst[:, :, :, 192:224], src[:, :, 2:4, :]
                    )

            def qpad_fill(t):
                sl = ts(t, 512)
                for h in range(HEADS):
                    nc.vector.tensor_copy(
                        q_pad[h][32 * h : 32 * h + 32, sl],
                        q_stage[32 * h : 32 * h + 32, sl],
                    )

            LN_FNS = (ln_A, ln_B, ln_C, ln_C2, ln_D, ln_E2)

            def ln_slot_schedule():
                """s-tiles 0-2 fully in the prologue (phase-major so the
                three tiles pipeline across engines); tiles 3-7 interleave
                into i-tile 0 COMPRESSED toward its start: the early exp
                stream is input-starved anyway, so the LN ACT/DVE work
                fills bubbles there and late i-tile 0 runs clean. Deadline:
                k/v/q(t) (slot E) before sims(4t), emitted at jc 4t-1."""
                sched = {}
                for fn, jc in zip(LN_FNS, (0, 1, 2, 4, 5, 6)):
                    sched.setdefault(jc, []).append((fn, 2))
                for t in range(3, 8):
                    e = min(2 * t + 6, 4 * t - 2)
                    offs = (2 * t - 6, 2 * t - 3, 2 * t, min(2 * t + 3, e - 3),
                            e - 1, e)
                    for fn, jc in zip(LN_FNS, offs):
                        sched.setdefault(jc, []).append((fn, t))
                for i in range(1, 4):
                    sched.setdefault(2 * i - 1, []).append((vaug_ones, i))
                # i-tile 1's q pads fill in i-tile 0's clean late zone
                sched.setdefault(20, []).append((qpad_fill, 1))
                return sched

            # ---------------- attention ----------------
            def make_tail(it, pairS, on_act):
                """Normalize+project closures run against the SBUF spill,
                sprinkled into the next i-tile's jc loop (off critical
                path). rec rows: 0 = even-head 1/denom, 32 = odd-head;
                cols 0:512 pairA, 512:1024 pairB. The final i-tile has
                nothing to hide under, so its reciprocals run as ACT
                ln/exp (~4x lower latency than DVE InstReciprocal)."""
                isl = ts(it, TI)
                st = {}

                def t_rec():
                    if on_act:
                        lnt = sm_pool.tile([1, 1024], dt, tag="lnt")
                        lnt2 = sm_pool.tile([1, 1024], dt, tag="lnt2")
                        nc.scalar.activation(lnt[:], pairS[32:33, :], F.Ln)
                        nc.scalar.activation(
                            rec[0:1, :], lnt[:], F.Exp, scale=-1.0
                        )
                        nc.scalar.activation(lnt2[:], pairS[96:97, :], F.Ln)
                        nc.scalar.activation(
                            rec[32:33, :], lnt2[:], F.Exp, scale=-1.0
                        )
                    else:
                        nc.vector.reciprocal(rec[0:1, :], pairS[32:33, :])
                        nc.vector.reciprocal(rec[32:33, :], pairS[96:97, :])

                def t_catA():
                    bcy = pools["duo"].tile([128, 1024], dt, tag="duo")
                    st["bc"] = bcy
                    st["yp"] = bcy
                    bc = bcy[0:97, 0:512]
                    nc.tensor.matmul(bc, ones2[0:33, :], rec[0:33, 0:512])
                    nc.vector.tensor_tensor(
                        catA[0:33, :], pairS[0:33, 0:512], bc[0:33, :], Op.mult
                    )
                    nc.vector.tensor_tensor(
                        catA[64:97, :], pairS[64:97, 0:512], bc[64:97, :],
                        Op.mult,
                    )

                def t_catB():
                    bc = st["bc"][0:97, 0:512]
                    nc.tensor.matmul(
                        st["yp"][:, 512:1024], woa[:, :], catA[0:97, :],
                        start=True, stop=False,
                    )
                    nc.tensor.matmul(bc, ones2[0:33, :], rec[0:33, 512:1024])
                    nc.vector.tensor_tensor(
                        catB[0:33, :], pairS[0:33, 512:1024], bc[0:33, :],
                        Op.mult,
                    )
                    nc.vector.tensor_tensor(
                        catB[64:97, :], pairS[64:97, 512:1024], bc[64:97, :],
                        Op.mult,
                    )

                def t_y():
                    yp = st["yp"][:, 512:1024]
                    nc.tensor.matmul(
                        yp, wob[:, :], catB[0:97, :], start=False, stop=True
                    )
                    ysb = ysb_pool.tile([128, TI], dt, tag="ysb")
                    nc.vector.tensor_scalar(
                        ysb[:], yp, bo[:, 0:1], None, Op.add
                    )
                    nc.sync.dma_start(y_d[:, isl], ysb[:])

                return [t_rec, t_catA, t_catB, t_y]

            def run_it(it, ln_sched, tail_pieces):
                isl = ts(it, TI)
                pairA = pair_pool.tile([128, TI], dt, tag="pair")
                pairB = pair_pool.tile([128, TI], dt, tag="pair")

                def emit_sims(jc):
                    ksl = ts(jc, JCHUNK)
                    duoX = pools["duo"].tile([128, 1024], dt, tag="duo")
                    nc.tensor.matmul(duoX[:, 0:512], k_sb[:, ksl], q_pad[0][:, isl])
                    nc.tensor.matmul(duoX[:, 512:1024], k_sb[:, ksl], q_pad[2][:, isl])
                    duoY = pools["duo"].tile([128, 1024], dt, tag="duo")
                    nc.tensor.matmul(duoY[:, 0:512], k_sb[:, ksl], q_pad[1][:, isl])
                    nc.tensor.matmul(duoY[:, 512:1024], k_sb[:, ksl], q_pad[3][:, isl])
                    return duoX, duoY

                duoX, duoY = emit_sims(0)
                for jc in range(NJC):
                    st, sp_ = jc == 0, jc == NJC - 1
                    vbase = jc * 512
                    expX = expo_pool.tile([128, 1024], dtb, tag="expo")
                    nc.scalar.activation(expX[:], duoX[:], F.Exp)
                    expY = expo_pool.tile([128, 1024], dtb, tag="expo")
                    nc.scalar.activation(expY[:], duoY[:], F.Exp)
                    if jc + 1 < NJC:
                        duoX, duoY = emit_sims(jc + 1)
                    nc.tensor.matmul(
                        pairA[:, :], vaug[:, vbase : vbase + 128],
                        expX[:, 0:512],
                        start=st, stop=False, skip_group_check=True,
                    )
                    nc.tensor.matmul(
                        pairA[:, :], vaug[:, vbase + 128 : vbase + 256],
                        expX[:, 512:1024],
                        start=False, stop=sp_, skip_group_check=True,
                    )
                    nc.tensor.matmul(
                        pairB[:, :], vaug[:, vbase + 256 : vbase + 384],
                        expY[:, 0:512],
                        start=st, stop=False, skip_group_check=True,
                    )
                    nc.tensor.matmul(
                        pairB[:, :], vaug[:, vbase + 384 : vbase + 512],
                        expY[:, 512:1024],
                        start=False, stop=sp_, skip_group_check=True,
                    )
                    # LN slots and tail pieces at the loop BOTTOM: this jc's
                    # avs were just emitted, so they don't (coarsely) wait
                    # on this slot's DVE work.
                    for fn, t in ln_sched.get(jc, ()):
                        fn(t)
                    # t_rec's ACT ln/exp ops ride the exp stream (~5us);
                    # the PE-visible bc/y pieces go late enough that the
                    # reciprocals are done when the PE queue reaches them.
                    if tail_pieces and jc in (1, 9, 11, 13, 15):
                        i = (1, 9, 11, 13, 15).index(jc)
                        if i < len(tail_pieces):
                            tail_pieces[i]()
                # spill pairs to SBUF: frees the pair banks after 2 quick
                # DVE copies; the tail runs later against the copy.
                pairS = spill_pool.tile([128, 1024], dt, tag="pairS")
                nc.vector.tensor_copy(pairS[:, 0:512], pairA[:])
                nc.vector.tensor_copy(pairS[:, 512:1024], pairB[:])
                return make_tail(it, pairS, on_act=True)

            sched = ln_slot_schedule()
            with (
                tc.tile_pool(name="lnps", bufs=2, space="PSUM") as lnp0,
                tc.tile_pool(name="duo0", bufs=2, space="PSUM") as duo0,
            ):
                pools["lnps"] = lnp0
                pools["duo"] = duo0
                vaug_ones(0)
                for fn in LN_FNS:
                    for t in range(2):
                        fn(t)
                qpad_fill(0)
                tail = run_it(0, sched, None)
            with tc.tile_pool(name="duo1", bufs=3, space="PSUM") as duo1:
                pools["duo"] = duo1
                for it in range(1, NIT):
                    if it + 1 < NIT:
                        tail.append(lambda t=it + 1: qpad_fill(t))
                    tail = run_it(it, {}, tail)
                for piece in tail:
                    piece()

    nc.compile()
    return nc


def _get_program():
    global _PROGRAM
    if _PROGRAM is None:
        _PROGRAM = _build_program()
    return _PROGRAM


def _prep_inputs(x, g, b, w_qkv, w_out, b_out):
    """Host-side sharding + weight folding. All tiny except x slicing."""
    f32 = np.float32
    x = np.asarray(x, f32).reshape(B, C, S)
    g_ = np.asarray(g, f32).reshape(C)
    b_ = np.asarray(b, f32).reshape(C)
    w_qkv = np.asarray(w_qkv, f32)
    w_out = np.asarray(w_out, f32)
    b_out = np.asarray(b_out, f32)

    import ml_dtypes

    bf16 = ml_dtypes.bfloat16
    scale = DIM_HEAD ** -0.5
    wg = w_qkv * g_[None, :]
    bias_qkv = w_qkv @ b_
    hid = HEADS * DIM_HEAD  # 128
    wq_t = np.ascontiguousarray((wg[0:hid] * scale).T).astype(bf16)
    wk_t = np.ascontiguousarray(wg[hid : 2 * hid].T).astype(bf16)
    wv_t = np.ascontiguousarray(wg[2 * hid : 3 * hid].T).astype(bf16)
    bias_q = np.ascontiguousarray((bias_qkv[0:hid] * scale).reshape(128, 1))
    # bias_k is dropped: it shifts all logits of a query equally and
    # softmax is shift-invariant (exact). bias_v folds exactly into the
    # output bias (attention rows sum to 1).
    bias_v = bias_qkv[2 * hid : 3 * hid]

    wo_t = w_out.T  # [hd, o]
    wo_a = np.zeros((97, 128), f32)
    wo_b = np.zeros((97, 128), f32)
    wo_a[0:32] = wo_t[0:32]     # head 0
    wo_a[64:96] = wo_t[64:96]   # head 2
    wo_b[0:32] = wo_t[32:64]    # head 1
    wo_b[64:96] = wo_t[96:128]  # head 3
    bias_o = np.ascontiguousarray((b_out + w_out @ bias_v).reshape(128, 1))

    shared = {
        "wq_t": wq_t,
        "wk_t": wk_t,
        "wv_t": wv_t,
        "wo_a": wo_a,
        "wo_b": wo_b,
        "bias_q": bias_q,
        "bias_o": bias_o,
        "zeros": np.zeros((128, 4096), bf16),
    }
    in_maps = []
    for core in range(N_CORES):
        bb, half = core // 2, core % 2
        if half == 0:
            xc = x[bb]
        else:
            xc = np.concatenate([x[bb][:, HALF:], x[bb][:, :HALF]], axis=1)
        m = {"x": np.ascontiguousarray(xc)}
        m.update(shared)
        in_maps.append(m)
    return in_maps


def _run(inputs, trace=False):
    from concourse.bass_utils import run_bass_kernel_spmd

    nc = _get_program()
    in_maps = _prep_inputs(**inputs)
    res = run_bass_kernel_spmd(
        nc, in_maps, core_ids=list(range(N_CORES)), trace=trace
    )
    y = np.empty((B, C, S), np.float32)
    for core in range(N_CORES):
        bb, half = core // 2, core % 2
        yc = res.results[core]["y"]
        if half == 0:
            y[bb][:, :HALF] = yc
        else:
            y[bb][:, HALF:] = yc
    return y.reshape(B, C, H, W), res


def kernel(x, g, b, w_qkv, w_out, b_out):
    out, _ = _run(
        {"x": x, "g": g, "b": b, "w_qkv": w_qkv, "w_out": w_out, "b_out": b_out}
    )
    return out
